# revision 16
# baseline (speedup 1.0000x reference)
"""Multi-head attention (EMB=512, HEADS=8, x:(4,2048,512)) on 8 Trainium2 cores.

Sharding: zero-collective split — core c handles batch c//2, query rows
(c%2)*1024..(c%2+1)*1024, ALL heads.  K/V projections for the full batch are
computed redundantly on the 2 cores sharing a batch (no collectives).

v2 changes vs v1:
  - Q/K projections in fp8e4m3 with DoubleRow matmuls (2 k-tiles per
    instruction at 2 rows/cycle): projection PE time halved.  Error budget
    (checked offline): Q/K fp8 perturbs logits by ~0.003 abs -> negligible.
  - parallel DMA queues (x on sync, weights on scalar, biases on gpsimd),
    x chunked by 512-token groups: PE starts at ~1.5us.
  - eager per-head-pair normalization: softmax denominator comes off the
    PV accumulator row 64 (single shared ones-column), reciprocal runs
    right after each head pair drains, broadcast sel-matmul per et —
    removes the end-of-chunk norm stall.
  - exp split: ScalarE activation for most g-groups, DVE Schraudolph
    bit-trick (bf16 bits = int16(x*128/ln2 + (127-C)*128)) for g in
    {3,6} (+{1} on alternating (hp,c)), balancing ACT/DVE at ~100us each.
"""

import sys
import os
import math

for _p in ("/opt/trn_rl_repo", "/root/.axon_site/_ro/trn_rl_repo"):
    if os.path.isdir(_p) and _p not in sys.path:
        sys.path.append(_p)

import numpy as np
import ml_dtypes

EMB = 512
HEADS = 8
D = 64  # head dim
B = 4
N = 2048  # keys / tokens per batch
HALF = 1024  # queries per core
P = 128
NCORES = 8
KT4 = EMB // P  # 4 contraction tiles
SCALE = float(1.0 / np.sqrt(np.float32(EMB)))

# Schraudolph exp producing bf16 bit patterns via int16:
#   bf16_bits(exp(s*SCALE)) ~= int16(s * (SCALE*128/ln2) + (127 - C)*128)
SCH_A = SCALE * 128.0 / math.log(2.0)
SCH_B = 127.0 * 128.0 - 0.045 * 128.0

_CACHE = {}


def _dve_gs(hp, c):
    # c=0: DVE is busy with projection bias-adds and V drains -> light exp.
    # c=1: projections done -> even ACT/DVE exp split.  The final unit
    # keeps g7 on ACT so the tail-critical PV drain isn't queued on DVE.
    if c == 1:
        return (1, 3, 5, 7)
    return (3, 6)


def _build_program():
    from concourse import bacc
    import concourse.mybir as mybir
    import concourse.tile as tile
    from contextlib import ExitStack

    dt = mybir.dt.float32
    f32r = mybir.dt.float32r
    bf16 = mybir.dt.bfloat16
    f8 = mybir.dt.float8e4
    i16 = mybir.dt.int16
    DR = mybir.MatmulPerfMode.DoubleRow
    nc = bacc.Bacc("TRN2", target_bir_lowering=False)

    xT_d = nc.dram_tensor("xT", [P, KT4, N], bf16, kind="ExternalInput")
    x8_d = nc.dram_tensor("x8", [P, KT4, N], f8, kind="ExternalInput")
    wq_d = nc.dram_tensor("wq", [P, KT4, EMB], f8, kind="ExternalInput")
    wk_d = nc.dram_tensor("wk", [P, KT4, EMB], f8, kind="ExternalInput")
    wv_d = nc.dram_tensor("wv", [P, KT4, EMB], bf16, kind="ExternalInput")
    wo_d = nc.dram_tensor("wo", [P, KT4, EMB], bf16, kind="ExternalInput")
    bq_d = nc.dram_tensor("bq2", [P, KT4], dt, kind="ExternalInput")
    bk_d = nc.dram_tensor("bk2", [P, KT4], dt, kind="ExternalInput")
    bvr_d = nc.dram_tensor("bvr", [P, EMB], dt, kind="ExternalInput")
    bor_d = nc.dram_tensor("bor", [P, EMB], dt, kind="ExternalInput")
    sel_d = nc.dram_tensor("sel", [HEADS, KT4, P], bf16, kind="ExternalInput")
    y_d = nc.dram_tensor("y", [HALF, EMB], bf16, kind="ExternalOutput")

    Exp = mybir.ActivationFunctionType.Exp
    mult = mybir.AluOpType.mult
    add = mybir.AluOpType.add

    with tile.TileContext(nc) as tc, ExitStack() as ctx:
        ptp = ctx.enter_context(tc.tile_pool(name="ptp", bufs=8))
        wp = ctx.enter_context(tc.tile_pool(name="wp", bufs=1))
        pers = ctx.enter_context(tc.tile_pool(name="pers", bufs=1))
        yp = ctx.enter_context(tc.tile_pool(name="yp", bufs=2))
        # PSUM: tag "s" 3 x [128,1024] slots (6 banks) + tag "pv" 2 x 1 bank
        ps = ctx.enter_context(tc.tile_pool(name="ps", bufs=3, space="PSUM"))

        # ---- SBUF tiles ----
        xt = pers.tile([P, KT4, N], bf16, name="xt")
        x8 = pers.tile([P, KT4, N], f8, name="x8")
        wq_s = wp.tile([P, KT4, EMB], f8, name="wqs", tag="wqs")
        wk_s = wp.tile([P, KT4, EMB], f8, name="wks", tag="wks")
        wv_s = wp.tile([P, KT4, EMB], bf16, name="wvs", tag="wvs")
        wo_s = wp.tile([P, KT4, EMB], bf16, name="wos", tag="wos")
        bq_s = pers.tile([P, KT4], dt, name="bqs")
        bk_s = pers.tile([P, KT4], dt, name="bks")
        bvr_s = pers.tile([P, HEADS, D], dt, name="bvrs")
        bor_s = pers.tile([P, EMB], dt, name="bors")
        sel_s = pers.tile([HEADS, KT4, P], bf16, name="sels")

        QT = pers.tile([P, KT4, HALF], bf16, name="QT")
        KTt = pers.tile([P, KT4, N], bf16, name="KTt")
        Vb = pers.tile([P, 16, HEADS, D + 8], bf16, name="Vb")
        outT = pers.tile([P, KT4, HALF], bf16, name="outT")
        sums = pers.tile([64 + HEADS, HALF], dt, name="sums")
        ones1 = pers.tile([1, P], bf16, name="ones1")
        bor1 = pers.tile([1, EMB], bf16, name="bor1")
        sums0 = pers.tile([HEADS, HALF], dt, name="sums0")
        rsum = pers.tile([HEADS, HALF], dt, name="rsum")
        rsumr = pers.tile([HEADS, HALF], bf16, name="rsumr")

        # ---- DMA: x on sync queue, weights on scalar queue, rest gpsimd
        nc.scalar.dma_start(wq_s[:], wq_d[:])
        nc.scalar.dma_start(wk_s[:], wk_d[:])
        for ch in range(4):
            sl = slice(ch * 512, (ch + 1) * 512)
            nc.sync.dma_start(x8[:, :, sl], x8_d[:, :, sl])
        nc.scalar.dma_start(wv_s[:], wv_d[:])
        for ch in range(4):
            sl = slice(ch * 512, (ch + 1) * 512)
            nc.sync.dma_start(xt[:, :, sl], xT_d[:, :, sl])
        nc.scalar.dma_start(wo_s[:], wo_d[:])
        nc.gpsimd.dma_start(bq_s[:], bq_d[:])
        nc.gpsimd.dma_start(bk_s[:], bk_d[:])
        nc.gpsimd.dma_start(bvr_s[:], bvr_d.ap().rearrange("p (h d) -> p h d", d=D))
        nc.gpsimd.dma_start(bor_s[:], bor_d[:])
        nc.gpsimd.dma_start(sel_s[:], sel_d[:])

        # per-head one-hot indicator columns: PV lands head h's softmax
        # denominator on PSUM partition 64+h (32-aligned drains)
        nc.vector.memset(Vb[:, :, :, D:D + 8], 0.0)
        for h in range(HEADS):
            nc.vector.memset(Vb[:, :, h, D + h], 1.0)
        nc.vector.memset(ones1[:], 1.0)
        nc.vector.memset(sums[64:64 + HEADS, :], 0.0)
        nc.vector.tensor_copy(bor1[:], bor_s[0:1, :])

        def emit_q(c):
            # QT[:, jt, c*512:(c+1)*512] for all jt; fp8 DoubleRow.
            # two jt-projections share one [128,1024] PSUM slot
            for jtp in range(2):
                pq2 = ps.tile([P, 2, 512], dt, tag="s", name=f"pq{jtp}{c}")
                for half in range(2):
                    jt = 2 * jtp + half
                    for kp in range(2):
                        nc.tensor.matmul(
                            pq2[:, half],
                            lhsT=wq_s[:, 2 * kp:2 * kp + 2, jt * P:(jt + 1) * P],
                            rhs=x8[:, 2 * kp:2 * kp + 2, c * 512:(c + 1) * 512],
                            start=kp == 0,
                            stop=kp == 1,
                            perf_mode=DR,
                        )
                    nc.scalar.activation(
                        QT[:, jt, c * 512:(c + 1) * 512], pq2[:, half],
                        mybir.ActivationFunctionType.Identity,
                        bias=bq_s[:, jt:jt + 1],
                    )

        def emit_k(kc):
            for jtp in range(2):
                pk2 = ps.tile([P, 2, 512], dt, tag="s", name=f"pk{jtp}{kc}")
                for half in range(2):
                    jt = 2 * jtp + half
                    for kp in range(2):
                        nc.tensor.matmul(
                            pk2[:, half],
                            lhsT=wk_s[:, 2 * kp:2 * kp + 2, jt * P:(jt + 1) * P],
                            rhs=x8[:, 2 * kp:2 * kp + 2, kc * 512:(kc + 1) * 512],
                            start=kp == 0,
                            stop=kp == 1,
                            perf_mode=DR,
                        )
                    nc.vector.tensor_scalar_add(
                        KTt[:, jt, kc * 512:(kc + 1) * 512], pk2[:, half],
                        bk_s[:, jt:jt + 1]
                    )

        def emit_v(vc):
            for tp in range(2):
                pv2 = ps.tile([P, 2, 512], dt, tag="s", name=f"pvv{vc}{tp}")
                for half in range(2):
                    t = 4 * vc + 2 * tp + half
                    for kt in range(KT4):
                        nc.tensor.matmul(
                            pv2[:, half],
                            lhsT=xt[:, kt, t * P:(t + 1) * P],
                            rhs=wv_s[:, kt, :],
                            start=kt == 0,
                            stop=kt == KT4 - 1,
                        )
                    nc.vector.tensor_tensor(
                        Vb[:, t, :, 0:D],
                        pv2[:, half].rearrange("p (h d) -> p h d", d=D),
                        bvr_s[:],
                        add,
                    )

        pv_acc = {}

        def emit_attn_quarter(hp, c, gp):
            hA, hB = 2 * hp, 2 * hp + 1
            jt = hp
            dve_gs = _dve_gs(hp, c)
            if gp == 0:
                pv_acc[0] = ps.tile([D + 8, 512], dt, tag="pv", bufs=2,
                                    name=f"pvA{hp}{c}")
                pv_acc[1] = ps.tile([D + 8, 512], dt, tag="pv", bufs=2,
                                    name=f"pvB{hp}{c}")
            pvA, pvB = pv_acc[0], pv_acc[1]
            for g in (2 * gp, 2 * gp + 1):
                sA = ps.tile([P, 1024], dt, tag="s", name=f"sA{hp}{c}{g}")
                sB = ps.tile([P, 1024], dt, tag="s", name=f"sB{hp}{c}{g}")
                for tt in range(2):
                    t = 2 * g + tt
                    nc.tensor.matmul(
                        sA[:, tt * 512:(tt + 1) * 512],
                        lhsT=KTt[0:D, jt, t * P:(t + 1) * P],
                        rhs=QT[0:D, jt, c * 512:(c + 1) * 512],
                        start=True,
                        stop=True,
                    )
                    nc.tensor.matmul(
                        sB[:, tt * 512:(tt + 1) * 512],
                        lhsT=KTt[D:P, jt, t * P:(t + 1) * P],
                        rhs=QT[D:P, jt, c * 512:(c + 1) * 512],
                        start=True,
                        stop=True,
                    )
                ptA = ptp.tile([P, 1024], bf16, tag="pt", name=f"ptA{hp}{c}{g}")
                ptB = ptp.tile([P, 1024], bf16, tag="pt", name=f"ptB{hp}{c}{g}")
                if g in dve_gs:
                    nc.vector.tensor_scalar(
                        ptA[:].bitcast(i16), sA[:], SCH_A, SCH_B, mult, add
                    )
                    nc.vector.tensor_scalar(
                        ptB[:].bitcast(i16), sB[:], SCH_A, SCH_B, mult, add
                    )
                else:
                    nc.scalar.activation(ptA[:], sA[:], Exp, scale=SCALE)
                    nc.scalar.activation(ptB[:], sB[:], Exp, scale=SCALE)
                for tt in range(2):
                    t = 2 * g + tt
                    nc.tensor.matmul(
                        pvA[:],
                        lhsT=Vb[:, t, hA, :],
                        rhs=ptA[:, tt * 512:(tt + 1) * 512],
                        start=t == 0,
                        stop=t == 15,
                    )
                    nc.tensor.matmul(
                        pvB[:],
                        lhsT=Vb[:, t, hB, :],
                        rhs=ptB[:, tt * 512:(tt + 1) * 512],
                        start=t == 0,
                        stop=t == 15,
                    )
            if gp == 3:
                for pv_, h in ((pvA, hA), (pvB, hB)):
                    po = (h % 2) * D
                    nc.scalar.copy(
                        outT[po:po + D, h // 2, c * 512:(c + 1) * 512], pv_[0:D, :]
                    )
                for pv_, h in ((pvA, hA), (pvB, hB)):
                    nc.vector.tensor_tensor(
                        sums[64:64 + HEADS, c * 512:(c + 1) * 512],
                        sums[64:64 + HEADS, c * 512:(c + 1) * 512],
                        pv_[D:D + 8, :],
                        add,
                    )

        def emit_norm(c):
            # outT[:, :, c-chunk] *= broadcast(1/sums) via PE sel matmul.
            # reciprocal_approx_fast is broken at partition base 64 - base 0
            sl = slice(c * 512, (c + 1) * 512)
            nc.vector.tensor_copy(sums0[:, sl], sums[64:64 + HEADS, sl])
            nc.vector.reciprocal_approx_fast(rsum[:, sl], sums0[:, sl])
            nc.vector.tensor_copy(rsumr[:, sl], rsum[:, sl])
            for et in range(KT4):
                pr = ps.tile([P, 512], dt, tag="s", name=f"pr{c}{et}")
                nc.tensor.matmul(
                    pr[:],
                    lhsT=sel_s[:, et, :],
                    rhs=rsumr[:, sl],
                    start=True,
                    stop=True,
                )
                nc.vector.tensor_tensor(
                    outT[:, et, sl],
                    outT[:, et, sl],
                    pr[:],
                    mult,
                )

        def emit_out(m):
            py = ps.tile([P, 512], dt, tag="s", name=f"py{m}")
            for et in range(KT4):
                nc.tensor.matmul(
                    py[:],
                    lhsT=outT[:, et, m * P:(m + 1) * P],
                    rhs=wo_s[:, et, :],
                    start=et == 0,
                    stop=et == KT4 - 1,
                )
            yt = yp.tile([P, 512], bf16, tag="y", name=f"yt{m}")
            nc.vector.tensor_tensor(yt[:], py[:], bor_s[:], add)
            ydma = (nc.sync, nc.scalar)
            ydma[m % 2].dma_start(y_d[m * P:(m + 1) * P, :], yt[:])

        # ---- emission schedule ----
        emit_q(0)
        emit_k(0)
        emit_k(1)
        emit_v(0)
        emit_v(1)
        emit_attn_quarter(0, 0, 0)
        emit_attn_quarter(0, 0, 1)
        emit_k(2)
        emit_v(2)
        emit_attn_quarter(0, 0, 2)
        emit_k(3)
        emit_v(3)
        emit_attn_quarter(0, 0, 3)
        emit_q(1)
        for hp in (1, 2, 3):
            for gp in range(4):
                emit_attn_quarter(hp, 0, gp)
        emit_norm(0)
        emit_attn_quarter(0, 1, 0)
        emit_attn_quarter(0, 1, 1)
        emit_attn_quarter(0, 1, 2)
        emit_out(0)
        emit_attn_quarter(0, 1, 3)
        emit_out(1)
        emit_attn_quarter(1, 1, 0)
        emit_attn_quarter(1, 1, 1)
        emit_attn_quarter(1, 1, 2)
        emit_out(2)
        emit_attn_quarter(1, 1, 3)
        emit_out(3)
        for gp in range(4):
            emit_attn_quarter(2, 1, gp)
        for gp in range(4):
            emit_attn_quarter(3, 1, gp)
        emit_norm(1)
        for m in (4, 5, 6, 7):
            emit_out(m)

    nc.finalize()
    return nc


def _get_program():
    if "nc" not in _CACHE:
        _CACHE["nc"] = _build_program()
    return _CACHE["nc"]


def _host_inputs(x, Wq, bq, Wk, bk, Wv, bv, Wo, bo):
    f32 = np.float32
    bf = ml_dtypes.bfloat16
    f8 = ml_dtypes.float8_e4m3

    def wprep(W, dtype):
        # [EMB_in, EMB_out] -> [P, KT4, EMB_out] (kt-tiled transpose)
        wT = np.asarray(W, f32).T.reshape(KT4, P, EMB)
        return np.ascontiguousarray(wT.transpose(1, 0, 2)).astype(dtype)

    wq2 = wprep(Wq, f8)
    wk2 = wprep(Wk, f8)
    wv2 = wprep(Wv, bf)
    wo2 = wprep(Wo, bf)
    bq2 = np.ascontiguousarray(np.asarray(bq, f32).reshape(KT4, P).T)
    bk2 = np.ascontiguousarray(np.asarray(bk, f32).reshape(KT4, P).T)
    bvr = np.ascontiguousarray(np.tile(np.asarray(bv, f32), (P, 1)))
    bor = np.ascontiguousarray(np.tile(np.asarray(bo, f32), (P, 1)))
    sel = np.zeros((HEADS, KT4, P), f32)
    for et in range(KT4):
        for m in range(P):
            sel[et * 2 + m // D, et, m] = 1.0

    shared = dict(wq=wq2, wk=wk2, wv=wv2, wo=wo2, bq2=bq2, bk2=bk2,
                  bvr=bvr, bor=bor, sel=sel.astype(bf))
    x = np.asarray(x, f32)
    in_maps = []
    for c in range(NCORES):
        b, hf = c // 2, c % 2
        xb = x[b]
        # queries first; key order is irrelevant as long as K and V agree
        xr = np.concatenate(
            [xb[hf * HALF:(hf + 1) * HALF], xb[(1 - hf) * HALF:(2 - hf) * HALF]], 0
        )
        xTf = np.ascontiguousarray(xr.T).reshape(KT4, P, N).transpose(1, 0, 2)
        xTc = np.ascontiguousarray(xTf)
        in_maps.append(dict(shared, xT=xTc.astype(bf), x8=xTc.astype(f8)))
    return in_maps


def kernel(x, Wq, bq, Wk, bk, Wv, bv, Wo, bo, _trace=False, _trace_cores=None):
    from concourse.bass_utils import run_bass_kernel_spmd

    nc = _get_program()
    in_maps = _host_inputs(x, Wq, bq, Wk, bk, Wv, bv, Wo, bo)
    res = run_bass_kernel_spmd(
        nc, in_maps, list(range(NCORES)), trace=_trace,
        trace_cores=_trace_cores,
    )
    out = np.empty((B, N, EMB), np.float32)
    for c in range(NCORES):
        b, hf = c // 2, c % 2
        out[b, hf * HALF:(hf + 1) * HALF] = np.asarray(
            res.results[c]["y"], np.float32
        )
    if _trace:
        _CACHE["last_results"] = res
    return out


# revision 18
# speedup vs baseline: 1.0121x; 1.0121x over previous
"""Multi-head attention (EMB=512, HEADS=8, x:(4,2048,512)) on 8 Trainium2 cores.

Sharding: zero-collective split — core c handles batch c//2, query rows
(c%2)*1024..(c%2+1)*1024, ALL heads.  K/V projections for the full batch are
computed redundantly on the 2 cores sharing a batch (no collectives).

v2 changes vs v1:
  - Q/K projections in fp8e4m3 with DoubleRow matmuls (2 k-tiles per
    instruction at 2 rows/cycle): projection PE time halved.  Error budget
    (checked offline): Q/K fp8 perturbs logits by ~0.003 abs -> negligible.
  - parallel DMA queues (x on sync, weights on scalar, biases on gpsimd),
    x chunked by 512-token groups: PE starts at ~1.5us.
  - eager per-head-pair normalization: softmax denominator comes off the
    PV accumulator row 64 (single shared ones-column), reciprocal runs
    right after each head pair drains, broadcast sel-matmul per et —
    removes the end-of-chunk norm stall.
  - exp split: ScalarE activation for most g-groups, DVE Schraudolph
    bit-trick (bf16 bits = int16(x*128/ln2 + (127-C)*128)) for g in
    {3,6} (+{1} on alternating (hp,c)), balancing ACT/DVE at ~100us each.
"""

import sys
import os
import math

for _p in ("/opt/trn_rl_repo", "/root/.axon_site/_ro/trn_rl_repo"):
    if os.path.isdir(_p) and _p not in sys.path:
        sys.path.append(_p)

import numpy as np
import ml_dtypes

EMB = 512
HEADS = 8
D = 64  # head dim
B = 4
N = 2048  # keys / tokens per batch
HALF = 1024  # queries per core
P = 128
NCORES = 8
KT4 = EMB // P  # 4 contraction tiles
SCALE = float(1.0 / np.sqrt(np.float32(EMB)))

# Schraudolph exp producing bf16 bit patterns via int16:
#   bf16_bits(exp(s*SCALE)) ~= int16(s * (SCALE*128/ln2) + (127 - C)*128)
SCH_A = SCALE * 128.0 / math.log(2.0)
SCH_B = 127.0 * 128.0 - 0.045 * 128.0

_CACHE = {}


def _dve_gs(hp, c):
    # c=0: DVE is busy with projection bias-adds and V drains -> light exp.
    # c=1: projections done -> even ACT/DVE exp split.  The final unit
    # keeps g7 on ACT so the tail-critical PV drain isn't queued on DVE.
    if c == 1:
        return (1, 3, 5, 7)
    return (3, 6)


def _build_program():
    from concourse import bacc
    import concourse.mybir as mybir
    import concourse.tile as tile
    from contextlib import ExitStack

    dt = mybir.dt.float32
    f32r = mybir.dt.float32r
    bf16 = mybir.dt.bfloat16
    f8 = mybir.dt.float8e4
    i16 = mybir.dt.int16
    DR = mybir.MatmulPerfMode.DoubleRow
    nc = bacc.Bacc("TRN2", target_bir_lowering=False)

    xT_d = nc.dram_tensor("xT", [P, KT4, N], bf16, kind="ExternalInput")
    x8_d = nc.dram_tensor("x8", [P, KT4, N], f8, kind="ExternalInput")
    wq_d = nc.dram_tensor("wq", [P, KT4, EMB], f8, kind="ExternalInput")
    wk_d = nc.dram_tensor("wk", [P, KT4, EMB], f8, kind="ExternalInput")
    wv_d = nc.dram_tensor("wv", [P, KT4, EMB], bf16, kind="ExternalInput")
    wo_d = nc.dram_tensor("wo", [P, KT4, EMB], bf16, kind="ExternalInput")
    bq_d = nc.dram_tensor("bq2", [P, KT4], dt, kind="ExternalInput")
    bk_d = nc.dram_tensor("bk2", [P, KT4], dt, kind="ExternalInput")
    bvr_d = nc.dram_tensor("bvr", [P, EMB], dt, kind="ExternalInput")
    bor_d = nc.dram_tensor("bor", [P, EMB], dt, kind="ExternalInput")
    sel_d = nc.dram_tensor("sel", [P, KT4, P], bf16, kind="ExternalInput")
    y_d = nc.dram_tensor("y", [HALF, EMB], bf16, kind="ExternalOutput")

    Exp = mybir.ActivationFunctionType.Exp
    mult = mybir.AluOpType.mult
    add = mybir.AluOpType.add

    with tile.TileContext(nc) as tc, ExitStack() as ctx:
        ptp = ctx.enter_context(tc.tile_pool(name="ptp", bufs=8))
        wp = ctx.enter_context(tc.tile_pool(name="wp", bufs=1))
        pers = ctx.enter_context(tc.tile_pool(name="pers", bufs=1))
        yp = ctx.enter_context(tc.tile_pool(name="yp", bufs=2))
        # PSUM: tag "s" 3 x [128,1024] slots (6 banks) + tag "pv" 2 x 1 bank
        ps = ctx.enter_context(tc.tile_pool(name="ps", bufs=3, space="PSUM"))

        # ---- SBUF tiles ----
        xt = pers.tile([P, KT4, N], bf16, name="xt")
        x8 = pers.tile([P, KT4, N], f8, name="x8")
        wq_s = wp.tile([P, KT4, EMB], f8, name="wqs", tag="wqs")
        wk_s = wp.tile([P, KT4, EMB], f8, name="wks", tag="wks")
        wv_s = wp.tile([P, KT4, EMB], bf16, name="wvs", tag="wvs")
        wo_s = wp.tile([P, KT4, EMB], bf16, name="wos", tag="wos")
        bq_s = pers.tile([P, KT4], dt, name="bqs")
        bk_s = pers.tile([P, KT4], dt, name="bks")
        bvr_s = pers.tile([P, HEADS, D], dt, name="bvrs")
        bor_s = pers.tile([P, EMB], dt, name="bors")
        sel_s = pers.tile([P, KT4, P], bf16, name="sels")

        QT = pers.tile([P, KT4, HALF], bf16, name="QT")
        KTt = pers.tile([P, KT4, N], bf16, name="KTt")
        Vb = pers.tile([P, 16, HEADS, D + 8], bf16, name="Vb")
        outT = pers.tile([P, KT4, HALF], bf16, name="outT")
        sums = pers.tile([64 + HEADS, HALF], dt, name="sums")
        ones1 = pers.tile([1, P], bf16, name="ones1")
        bor1 = pers.tile([1, EMB], bf16, name="bor1")
        sums0 = pers.tile([HEADS, HALF], dt, name="sums0")
        rsum = pers.tile([HEADS, HALF], dt, name="rsum")
        rsumr = pers.tile([P, HALF], bf16, name="rsumr")

        # ---- DMA: x on sync queue, weights on scalar queue, rest gpsimd
        nc.scalar.dma_start(wq_s[:], wq_d[:])
        nc.scalar.dma_start(wk_s[:], wk_d[:])
        for ch in range(4):
            sl = slice(ch * 512, (ch + 1) * 512)
            nc.sync.dma_start(x8[:, :, sl], x8_d[:, :, sl])
        nc.scalar.dma_start(wv_s[:], wv_d[:])
        for ch in range(4):
            sl = slice(ch * 512, (ch + 1) * 512)
            nc.sync.dma_start(xt[:, :, sl], xT_d[:, :, sl])
        nc.scalar.dma_start(wo_s[:], wo_d[:])
        nc.gpsimd.dma_start(bq_s[:], bq_d[:])
        nc.gpsimd.dma_start(bk_s[:], bk_d[:])
        nc.gpsimd.dma_start(bvr_s[:], bvr_d.ap().rearrange("p (h d) -> p h d", d=D))
        nc.gpsimd.dma_start(bor_s[:], bor_d[:])
        nc.gpsimd.dma_start(sel_s[:], sel_d[:])

        # per-head one-hot indicator columns: PV lands head h's softmax
        # denominator on PSUM partition 64+h (32-aligned drains)
        nc.vector.memset(Vb[:, :, :, D:D + 8], 0.0)
        for h in range(HEADS):
            nc.vector.memset(Vb[:, :, h, D + h], 1.0)
        nc.vector.memset(ones1[:], 1.0)
        nc.vector.memset(sums[64:64 + HEADS, :], 0.0)
        nc.vector.memset(rsumr[:], 0.0)
        nc.vector.tensor_copy(bor1[:], bor_s[0:1, :])

        def emit_q(c):
            # QT[:, jt, c*512:(c+1)*512] for all jt; fp8 DoubleRow.
            # two jt-projections share one [128,1024] PSUM slot
            for jtp in range(2):
                pq2 = ps.tile([P, 2, 512], dt, tag="s", name=f"pq{jtp}{c}")
                for half in range(2):
                    jt = 2 * jtp + half
                    for kp in range(2):
                        nc.tensor.matmul(
                            pq2[:, half],
                            lhsT=wq_s[:, 2 * kp:2 * kp + 2, jt * P:(jt + 1) * P],
                            rhs=x8[:, 2 * kp:2 * kp + 2, c * 512:(c + 1) * 512],
                            start=kp == 0,
                            stop=kp == 1,
                            perf_mode=DR,
                        )
                    nc.scalar.activation(
                        QT[:, jt, c * 512:(c + 1) * 512], pq2[:, half],
                        mybir.ActivationFunctionType.Identity,
                        bias=bq_s[:, jt:jt + 1],
                    )

        def emit_k(kc):
            for jtp in range(2):
                pk2 = ps.tile([P, 2, 512], dt, tag="s", name=f"pk{jtp}{kc}")
                for half in range(2):
                    jt = 2 * jtp + half
                    for kp in range(2):
                        nc.tensor.matmul(
                            pk2[:, half],
                            lhsT=wk_s[:, 2 * kp:2 * kp + 2, jt * P:(jt + 1) * P],
                            rhs=x8[:, 2 * kp:2 * kp + 2, kc * 512:(kc + 1) * 512],
                            start=kp == 0,
                            stop=kp == 1,
                            perf_mode=DR,
                        )
                    nc.vector.tensor_scalar_add(
                        KTt[:, jt, kc * 512:(kc + 1) * 512], pk2[:, half],
                        bk_s[:, jt:jt + 1]
                    )

        def emit_v(vc):
            for tp in range(2):
                pv2 = ps.tile([P, 2, 512], dt, tag="s", name=f"pvv{vc}{tp}")
                for half in range(2):
                    t = 4 * vc + 2 * tp + half
                    for kt in range(KT4):
                        nc.tensor.matmul(
                            pv2[:, half],
                            lhsT=xt[:, kt, t * P:(t + 1) * P],
                            rhs=wv_s[:, kt, :],
                            start=kt == 0,
                            stop=kt == KT4 - 1,
                        )
                    nc.vector.tensor_tensor(
                        Vb[:, t, :, 0:D],
                        pv2[:, half].rearrange("p (h d) -> p h d", d=D),
                        bvr_s[:],
                        add,
                    )

        pv_acc = {}

        def emit_attn_quarter(hp, c, gp):
            hA, hB = 2 * hp, 2 * hp + 1
            jt = hp
            dve_gs = _dve_gs(hp, c)
            if gp == 0:
                pv_acc[0] = ps.tile([D + 8, 512], dt, tag="pv", bufs=2,
                                    name=f"pvA{hp}{c}")
                pv_acc[1] = ps.tile([D + 8, 512], dt, tag="pv", bufs=2,
                                    name=f"pvB{hp}{c}")
            pvA, pvB = pv_acc[0], pv_acc[1]
            for g in (2 * gp, 2 * gp + 1):
                sA = ps.tile([P, 1024], dt, tag="s", name=f"sA{hp}{c}{g}")
                sB = ps.tile([P, 1024], dt, tag="s", name=f"sB{hp}{c}{g}")
                for tt in range(2):
                    t = 2 * g + tt
                    nc.tensor.matmul(
                        sA[:, tt * 512:(tt + 1) * 512],
                        lhsT=KTt[0:D, jt, t * P:(t + 1) * P],
                        rhs=QT[0:D, jt, c * 512:(c + 1) * 512],
                        start=True,
                        stop=True,
                    )
                    nc.tensor.matmul(
                        sB[:, tt * 512:(tt + 1) * 512],
                        lhsT=KTt[D:P, jt, t * P:(t + 1) * P],
                        rhs=QT[D:P, jt, c * 512:(c + 1) * 512],
                        start=True,
                        stop=True,
                    )
                ptA = ptp.tile([P, 1024], bf16, tag="pt", name=f"ptA{hp}{c}{g}")
                ptB = ptp.tile([P, 1024], bf16, tag="pt", name=f"ptB{hp}{c}{g}")
                if g in dve_gs:
                    nc.vector.tensor_scalar(
                        ptA[:].bitcast(i16), sA[:], SCH_A, SCH_B, mult, add
                    )
                    nc.vector.tensor_scalar(
                        ptB[:].bitcast(i16), sB[:], SCH_A, SCH_B, mult, add
                    )
                else:
                    nc.scalar.activation(ptA[:], sA[:], Exp, scale=SCALE)
                    nc.scalar.activation(ptB[:], sB[:], Exp, scale=SCALE)
                for tt in range(2):
                    t = 2 * g + tt
                    nc.tensor.matmul(
                        pvA[:],
                        lhsT=Vb[:, t, hA, :],
                        rhs=ptA[:, tt * 512:(tt + 1) * 512],
                        start=t == 0,
                        stop=t == 15,
                    )
                    nc.tensor.matmul(
                        pvB[:],
                        lhsT=Vb[:, t, hB, :],
                        rhs=ptB[:, tt * 512:(tt + 1) * 512],
                        start=t == 0,
                        stop=t == 15,
                    )
            if gp == 3:
                for pv_, h in ((pvA, hA), (pvB, hB)):
                    po = (h % 2) * D
                    nc.scalar.copy(
                        outT[po:po + D, h // 2, c * 512:(c + 1) * 512], pv_[0:D, :]
                    )
                for pv_, h in ((pvA, hA), (pvB, hB)):
                    nc.vector.tensor_tensor(
                        sums[64:64 + HEADS, c * 512:(c + 1) * 512],
                        sums[64:64 + HEADS, c * 512:(c + 1) * 512],
                        pv_[D:D + 8, :],
                        add,
                    )

        def emit_norm(c):
            # outT[:, :, c-chunk] *= broadcast(1/sums) via PE sel matmul.
            # reciprocal_approx_fast is broken at partition base 64 - base 0
            sl = slice(c * 512, (c + 1) * 512)
            nc.vector.tensor_copy(sums0[:, sl], sums[64:64 + HEADS, sl])
            nc.vector.reciprocal_approx_fast(rsum[:, sl], sums0[:, sl])
            nc.vector.tensor_copy(rsumr[0:HEADS, sl], rsum[:, sl])
            for et in range(KT4):
                pr = ps.tile([P, 512], dt, tag="s", name=f"pr{c}{et}")
                nc.tensor.matmul(
                    pr[:],
                    lhsT=sel_s[:, et, :],
                    rhs=rsumr[:, sl],
                    start=True,
                    stop=True,
                )
                nc.vector.tensor_tensor(
                    outT[:, et, sl],
                    outT[:, et, sl],
                    pr[:],
                    mult,
                )

        def emit_out(m):
            py = ps.tile([P, 512], dt, tag="s", name=f"py{m}")
            for et in range(KT4):
                nc.tensor.matmul(
                    py[:],
                    lhsT=outT[:, et, m * P:(m + 1) * P],
                    rhs=wo_s[:, et, :],
                    start=et == 0,
                    stop=et == KT4 - 1,
                )
            yt = yp.tile([P, 512], bf16, tag="y", name=f"yt{m}")
            nc.vector.tensor_tensor(yt[:], py[:], bor_s[:], add)
            ydma = (nc.sync, nc.scalar)
            ydma[m % 2].dma_start(y_d[m * P:(m + 1) * P, :], yt[:])

        # ---- emission schedule ----
        emit_q(0)
        emit_k(0)
        emit_k(1)
        emit_v(0)
        emit_v(1)
        emit_attn_quarter(0, 0, 0)
        emit_attn_quarter(0, 0, 1)
        emit_k(2)
        emit_v(2)
        emit_attn_quarter(0, 0, 2)
        emit_k(3)
        emit_v(3)
        emit_attn_quarter(0, 0, 3)
        emit_q(1)
        for hp in (1, 2, 3):
            for gp in range(4):
                emit_attn_quarter(hp, 0, gp)
        emit_norm(0)
        emit_attn_quarter(0, 1, 0)
        emit_attn_quarter(0, 1, 1)
        emit_attn_quarter(0, 1, 2)
        emit_out(0)
        emit_attn_quarter(0, 1, 3)
        emit_out(1)
        emit_attn_quarter(1, 1, 0)
        emit_attn_quarter(1, 1, 1)
        emit_attn_quarter(1, 1, 2)
        emit_out(2)
        emit_attn_quarter(1, 1, 3)
        emit_out(3)
        for gp in range(4):
            emit_attn_quarter(2, 1, gp)
        for gp in range(4):
            emit_attn_quarter(3, 1, gp)
        emit_norm(1)
        for m in (4, 5, 6, 7):
            emit_out(m)

    nc.finalize()
    return nc


def _get_program():
    if "nc" not in _CACHE:
        _CACHE["nc"] = _build_program()
    return _CACHE["nc"]


def _host_inputs(x, Wq, bq, Wk, bk, Wv, bv, Wo, bo):
    f32 = np.float32
    bf = ml_dtypes.bfloat16
    f8 = ml_dtypes.float8_e4m3

    def wprep(W, dtype):
        # [EMB_in, EMB_out] -> [P, KT4, EMB_out] (kt-tiled transpose)
        wT = np.asarray(W, f32).T.reshape(KT4, P, EMB)
        return np.ascontiguousarray(wT.transpose(1, 0, 2)).astype(dtype)

    wq2 = wprep(Wq, f8)
    wk2 = wprep(Wk, f8)
    wv2 = wprep(Wv, bf)
    wo2 = wprep(Wo, bf)
    bq2 = np.ascontiguousarray(np.asarray(bq, f32).reshape(KT4, P).T)
    bk2 = np.ascontiguousarray(np.asarray(bk, f32).reshape(KT4, P).T)
    bvr = np.ascontiguousarray(np.tile(np.asarray(bv, f32), (P, 1)))
    bor = np.ascontiguousarray(np.tile(np.asarray(bo, f32), (P, 1)))
    sel = np.zeros((P, KT4, P), f32)
    for et in range(KT4):
        for m in range(P):
            sel[et * 2 + m // D, et, m] = 1.0

    shared = dict(wq=wq2, wk=wk2, wv=wv2, wo=wo2, bq2=bq2, bk2=bk2,
                  bvr=bvr, bor=bor, sel=sel.astype(bf))
    x = np.asarray(x, f32)
    in_maps = []
    for c in range(NCORES):
        b, hf = c // 2, c % 2
        xb = x[b]
        # queries first; key order is irrelevant as long as K and V agree
        xr = np.concatenate(
            [xb[hf * HALF:(hf + 1) * HALF], xb[(1 - hf) * HALF:(2 - hf) * HALF]], 0
        )
        xTf = np.ascontiguousarray(xr.T).reshape(KT4, P, N).transpose(1, 0, 2)
        xTc = np.ascontiguousarray(xTf)
        in_maps.append(dict(shared, xT=xTc.astype(bf), x8=xTc.astype(f8)))
    return in_maps


def kernel(x, Wq, bq, Wk, bk, Wv, bv, Wo, bo, _trace=False, _trace_cores=None):
    from concourse.bass_utils import run_bass_kernel_spmd

    nc = _get_program()
    in_maps = _host_inputs(x, Wq, bq, Wk, bk, Wv, bv, Wo, bo)
    res = run_bass_kernel_spmd(
        nc, in_maps, list(range(NCORES)), trace=_trace,
        trace_cores=_trace_cores,
    )
    out = np.empty((B, N, EMB), np.float32)
    for c in range(NCORES):
        b, hf = c // 2, c % 2
        out[b, hf * HALF:(hf + 1) * HALF] = np.asarray(
            res.results[c]["y"], np.float32
        )
    if _trace:
        _CACHE["last_results"] = res
    return out


# revision 19
# speedup vs baseline: 1.0188x; 1.0066x over previous
"""Multi-head attention (EMB=512, HEADS=8, x:(4,2048,512)) on 8 Trainium2 cores.

Sharding: zero-collective split — core c handles batch c//2, query rows
(c%2)*1024..(c%2+1)*1024, ALL heads.  K/V projections for the full batch are
computed redundantly on the 2 cores sharing a batch (no collectives).

Optimizations vs the f32r baseline (251us -> ~212us):
  - all attention/PV/out matmuls in bf16; Q/K projections in fp8e4m3 with
    DoubleRow matmuls (2 k-tiles per instruction): projection PE time
    halved.  Error budget (validated offline and on HW): rel err ~4.8e-3
    vs 2e-2 tolerance.
  - exp split across two engines: ScalarE table-based Exp for most
    g-groups, DVE Schraudolph bit-trick for the rest
    (bf16 bits = int16(s * SCALE*128/ln2 + (127-0.045)*128), one
    tensor_scalar writing int16 into a bf16-tile bitcast).  c=0 units are
    ACT-heavy (DVE busy with projection bias-adds/V drains), c=1 units
    split evenly.
  - parallel DMA queues (x chunks on sync, weights on scalar, biases on
    gpsimd; y out on sync+scalar), x chunked by 512-token groups so
    projections start as soon as the first chunk lands.
  - c-major attention schedule: out-projection for query chunk 0 overlaps
    attention for chunk 1; K/V projections interleave with the first
    attention unit; PV drain copies on ScalarE (Identity/Copy share the
    exp activation table, so no table thrash).
  - y stored bf16 on device, upcast to fp32 on host.
"""

import sys
import os
import math

for _p in ("/opt/trn_rl_repo", "/root/.axon_site/_ro/trn_rl_repo"):
    if os.path.isdir(_p) and _p not in sys.path:
        sys.path.append(_p)

import numpy as np
import ml_dtypes

EMB = 512
HEADS = 8
D = 64  # head dim
B = 4
N = 2048  # keys / tokens per batch
HALF = 1024  # queries per core
P = 128
NCORES = 8
KT4 = EMB // P  # 4 contraction tiles
SCALE = float(1.0 / np.sqrt(np.float32(EMB)))

# Schraudolph exp producing bf16 bit patterns via int16:
#   bf16_bits(exp(s*SCALE)) ~= int16(s * (SCALE*128/ln2) + (127 - C)*128)
SCH_A = SCALE * 128.0 / math.log(2.0)
SCH_B = 127.0 * 128.0 - 0.045 * 128.0

_CACHE = {}


def _dve_gs(hp, c):
    # c=0: DVE is busy with projection bias-adds and V drains -> light exp.
    # c=1: projections done -> even ACT/DVE exp split.  The final unit
    # keeps g7 on ACT so the tail-critical PV drain isn't queued on DVE.
    if c == 1:
        return (1, 3, 5, 7)
    return (3, 6)


def _build_program():
    from concourse import bacc
    import concourse.mybir as mybir
    import concourse.tile as tile
    from contextlib import ExitStack

    dt = mybir.dt.float32
    f32r = mybir.dt.float32r
    bf16 = mybir.dt.bfloat16
    f8 = mybir.dt.float8e4
    i16 = mybir.dt.int16
    DR = mybir.MatmulPerfMode.DoubleRow
    nc = bacc.Bacc("TRN2", target_bir_lowering=False)

    xT_d = nc.dram_tensor("xT", [P, KT4, N], bf16, kind="ExternalInput")
    x8_d = nc.dram_tensor("x8", [P, KT4, N], f8, kind="ExternalInput")
    wq_d = nc.dram_tensor("wq", [P, KT4, EMB], f8, kind="ExternalInput")
    wk_d = nc.dram_tensor("wk", [P, KT4, EMB], f8, kind="ExternalInput")
    wv_d = nc.dram_tensor("wv", [P, KT4, EMB], bf16, kind="ExternalInput")
    wo_d = nc.dram_tensor("wo", [P, KT4, EMB], bf16, kind="ExternalInput")
    bq_d = nc.dram_tensor("bq2", [P, KT4], dt, kind="ExternalInput")
    bk_d = nc.dram_tensor("bk2", [P, KT4], dt, kind="ExternalInput")
    bvr_d = nc.dram_tensor("bvr", [P, EMB], dt, kind="ExternalInput")
    bor_d = nc.dram_tensor("bor", [P, EMB], dt, kind="ExternalInput")
    sel_d = nc.dram_tensor("sel", [P, KT4, P], bf16, kind="ExternalInput")
    y_d = nc.dram_tensor("y", [HALF, EMB], bf16, kind="ExternalOutput")

    Exp = mybir.ActivationFunctionType.Exp
    mult = mybir.AluOpType.mult
    add = mybir.AluOpType.add

    with tile.TileContext(nc) as tc, ExitStack() as ctx:
        ptp = ctx.enter_context(tc.tile_pool(name="ptp", bufs=8))
        wp = ctx.enter_context(tc.tile_pool(name="wp", bufs=1))
        pers = ctx.enter_context(tc.tile_pool(name="pers", bufs=1))
        yp = ctx.enter_context(tc.tile_pool(name="yp", bufs=2))
        # PSUM: tag "s" 3 x [128,1024] slots (6 banks) + tag "pv" 2 x 1 bank
        ps = ctx.enter_context(tc.tile_pool(name="ps", bufs=3, space="PSUM"))

        # ---- SBUF tiles ----
        xt = pers.tile([P, KT4, N], bf16, name="xt")
        x8 = pers.tile([P, KT4, N], f8, name="x8")
        wq_s = wp.tile([P, KT4, EMB], f8, name="wqs", tag="wqs")
        wk_s = wp.tile([P, KT4, EMB], f8, name="wks", tag="wks")
        wv_s = wp.tile([P, KT4, EMB], bf16, name="wvs", tag="wvs")
        wo_s = wp.tile([P, KT4, EMB], bf16, name="wos", tag="wos")
        bq_s = pers.tile([P, KT4], dt, name="bqs")
        bk_s = pers.tile([P, KT4], dt, name="bks")
        bvr_s = pers.tile([P, HEADS, D], dt, name="bvrs")
        bor_s = pers.tile([P, EMB], dt, name="bors")
        sel_s = pers.tile([P, KT4, P], bf16, name="sels")

        QT = pers.tile([P, KT4, HALF], bf16, name="QT")
        KTt = pers.tile([P, KT4, N], bf16, name="KTt")
        Vb = pers.tile([P, 16, HEADS, D + 8], bf16, name="Vb")
        outT = pers.tile([P, KT4, HALF], bf16, name="outT")
        sums = pers.tile([64 + HEADS, HALF], dt, name="sums")
        ones1 = pers.tile([1, P], bf16, name="ones1")
        bor1 = pers.tile([1, EMB], bf16, name="bor1")
        sums0 = pers.tile([HEADS, HALF], dt, name="sums0")
        rsum = pers.tile([HEADS, HALF], dt, name="rsum")
        rsumr = pers.tile([P, HALF], bf16, name="rsumr")

        # ---- DMA: x on sync queue, weights on scalar queue, rest gpsimd
        nc.scalar.dma_start(wq_s[:], wq_d[:])
        nc.scalar.dma_start(wk_s[:], wk_d[:])
        for ch in range(4):
            sl = slice(ch * 512, (ch + 1) * 512)
            nc.sync.dma_start(x8[:, :, sl], x8_d[:, :, sl])
        nc.scalar.dma_start(wv_s[:], wv_d[:])
        for ch in range(4):
            sl = slice(ch * 512, (ch + 1) * 512)
            nc.sync.dma_start(xt[:, :, sl], xT_d[:, :, sl])
        nc.scalar.dma_start(wo_s[:], wo_d[:])
        nc.gpsimd.dma_start(bq_s[:], bq_d[:])
        nc.gpsimd.dma_start(bk_s[:], bk_d[:])
        nc.gpsimd.dma_start(bvr_s[:], bvr_d.ap().rearrange("p (h d) -> p h d", d=D))
        nc.gpsimd.dma_start(bor_s[:], bor_d[:])
        nc.gpsimd.dma_start(sel_s[:], sel_d[:])

        # per-head one-hot indicator columns: PV lands head h's softmax
        # denominator on PSUM partition 64+h (32-aligned drains)
        nc.vector.memset(Vb[:, :, :, D:D + 8], 0.0)
        for h in range(HEADS):
            nc.vector.memset(Vb[:, :, h, D + h], 1.0)
        nc.vector.memset(ones1[:], 1.0)
        nc.vector.memset(sums[64:64 + HEADS, :], 0.0)
        nc.vector.memset(rsumr[:], 0.0)
        nc.vector.tensor_copy(bor1[:], bor_s[0:1, :])

        def emit_q(c):
            # QT[:, jt, c*512:(c+1)*512] for all jt; fp8 DoubleRow.
            # two jt-projections share one [128,1024] PSUM slot
            for jtp in range(2):
                pq2 = ps.tile([P, 2, 512], dt, tag="s", name=f"pq{jtp}{c}")
                for half in range(2):
                    jt = 2 * jtp + half
                    for kp in range(2):
                        nc.tensor.matmul(
                            pq2[:, half],
                            lhsT=wq_s[:, 2 * kp:2 * kp + 2, jt * P:(jt + 1) * P],
                            rhs=x8[:, 2 * kp:2 * kp + 2, c * 512:(c + 1) * 512],
                            start=kp == 0,
                            stop=kp == 1,
                            perf_mode=DR,
                        )
                    nc.scalar.activation(
                        QT[:, jt, c * 512:(c + 1) * 512], pq2[:, half],
                        mybir.ActivationFunctionType.Identity,
                        bias=bq_s[:, jt:jt + 1],
                    )

        def emit_k(kc):
            for jtp in range(2):
                pk2 = ps.tile([P, 2, 512], dt, tag="s", name=f"pk{jtp}{kc}")
                for half in range(2):
                    jt = 2 * jtp + half
                    for kp in range(2):
                        nc.tensor.matmul(
                            pk2[:, half],
                            lhsT=wk_s[:, 2 * kp:2 * kp + 2, jt * P:(jt + 1) * P],
                            rhs=x8[:, 2 * kp:2 * kp + 2, kc * 512:(kc + 1) * 512],
                            start=kp == 0,
                            stop=kp == 1,
                            perf_mode=DR,
                        )
                    nc.vector.tensor_scalar_add(
                        KTt[:, jt, kc * 512:(kc + 1) * 512], pk2[:, half],
                        bk_s[:, jt:jt + 1]
                    )

        def emit_v(vc):
            for tp in range(2):
                pv2 = ps.tile([P, 2, 512], dt, tag="s", name=f"pvv{vc}{tp}")
                for half in range(2):
                    t = 4 * vc + 2 * tp + half
                    for kt in range(KT4):
                        nc.tensor.matmul(
                            pv2[:, half],
                            lhsT=xt[:, kt, t * P:(t + 1) * P],
                            rhs=wv_s[:, kt, :],
                            start=kt == 0,
                            stop=kt == KT4 - 1,
                        )
                    nc.vector.tensor_tensor(
                        Vb[:, t, :, 0:D],
                        pv2[:, half].rearrange("p (h d) -> p h d", d=D),
                        bvr_s[:],
                        add,
                    )

        pv_acc = {}

        def emit_attn_quarter(hp, c, gp):
            hA, hB = 2 * hp, 2 * hp + 1
            jt = hp
            dve_gs = _dve_gs(hp, c)
            if gp == 0:
                pv_acc[0] = ps.tile([D + 8, 512], dt, tag="pv", bufs=2,
                                    name=f"pvA{hp}{c}")
                pv_acc[1] = ps.tile([D + 8, 512], dt, tag="pv", bufs=2,
                                    name=f"pvB{hp}{c}")
            pvA, pvB = pv_acc[0], pv_acc[1]
            for g in (2 * gp, 2 * gp + 1):
                sA = ps.tile([P, 1024], dt, tag="s", name=f"sA{hp}{c}{g}")
                sB = ps.tile([P, 1024], dt, tag="s", name=f"sB{hp}{c}{g}")
                for tt in range(2):
                    t = 2 * g + tt
                    nc.tensor.matmul(
                        sA[:, tt * 512:(tt + 1) * 512],
                        lhsT=KTt[0:D, jt, t * P:(t + 1) * P],
                        rhs=QT[0:D, jt, c * 512:(c + 1) * 512],
                        start=True,
                        stop=True,
                    )
                    nc.tensor.matmul(
                        sB[:, tt * 512:(tt + 1) * 512],
                        lhsT=KTt[D:P, jt, t * P:(t + 1) * P],
                        rhs=QT[D:P, jt, c * 512:(c + 1) * 512],
                        start=True,
                        stop=True,
                    )
                ptA = ptp.tile([P, 1024], bf16, tag="pt", name=f"ptA{hp}{c}{g}")
                ptB = ptp.tile([P, 1024], bf16, tag="pt", name=f"ptB{hp}{c}{g}")
                if g in dve_gs:
                    nc.vector.tensor_scalar(
                        ptA[:].bitcast(i16), sA[:], SCH_A, SCH_B, mult, add
                    )
                    nc.vector.tensor_scalar(
                        ptB[:].bitcast(i16), sB[:], SCH_A, SCH_B, mult, add
                    )
                else:
                    nc.scalar.activation(ptA[:], sA[:], Exp, scale=SCALE)
                    nc.scalar.activation(ptB[:], sB[:], Exp, scale=SCALE)
                for tt in range(2):
                    t = 2 * g + tt
                    nc.tensor.matmul(
                        pvA[:],
                        lhsT=Vb[:, t, hA, :],
                        rhs=ptA[:, tt * 512:(tt + 1) * 512],
                        start=t == 0,
                        stop=t == 15,
                    )
                    nc.tensor.matmul(
                        pvB[:],
                        lhsT=Vb[:, t, hB, :],
                        rhs=ptB[:, tt * 512:(tt + 1) * 512],
                        start=t == 0,
                        stop=t == 15,
                    )
            if gp == 3:
                for pv_, h in ((pvA, hA), (pvB, hB)):
                    po = (h % 2) * D
                    nc.scalar.copy(
                        outT[po:po + D, h // 2, c * 512:(c + 1) * 512], pv_[0:D, :]
                    )
                for pv_, h in ((pvA, hA), (pvB, hB)):
                    nc.vector.tensor_tensor(
                        sums[64:64 + HEADS, c * 512:(c + 1) * 512],
                        sums[64:64 + HEADS, c * 512:(c + 1) * 512],
                        pv_[D:D + 8, :],
                        add,
                    )

        def emit_norm(c):
            # outT[:, :, c-chunk] *= broadcast(1/sums) via PE sel matmul.
            # reciprocal_approx_fast is broken at partition base 64 - base 0
            sl = slice(c * 512, (c + 1) * 512)
            nc.vector.tensor_copy(sums0[:, sl], sums[64:64 + HEADS, sl])
            nc.vector.reciprocal_approx_fast(rsum[:, sl], sums0[:, sl])
            nc.vector.tensor_copy(rsumr[0:HEADS, sl], rsum[:, sl])
            for et in range(KT4):
                pr = ps.tile([P, 512], dt, tag="s", name=f"pr{c}{et}")
                nc.tensor.matmul(
                    pr[:],
                    lhsT=sel_s[:, et, :],
                    rhs=rsumr[:, sl],
                    start=True,
                    stop=True,
                )
                nc.vector.tensor_tensor(
                    outT[:, et, sl],
                    outT[:, et, sl],
                    pr[:],
                    mult,
                )

        def emit_out(m):
            py = ps.tile([P, 512], dt, tag="s", name=f"py{m}")
            for et in range(KT4):
                nc.tensor.matmul(
                    py[:],
                    lhsT=outT[:, et, m * P:(m + 1) * P],
                    rhs=wo_s[:, et, :],
                    start=et == 0,
                    stop=et == KT4 - 1,
                )
            yt = yp.tile([P, 512], bf16, tag="y", name=f"yt{m}")
            nc.vector.tensor_tensor(yt[:], py[:], bor_s[:], add)
            ydma = (nc.sync, nc.scalar)
            ydma[m % 2].dma_start(y_d[m * P:(m + 1) * P, :], yt[:])

        # ---- emission schedule ----
        emit_q(0)
        emit_k(0)
        emit_k(1)
        emit_v(0)
        emit_v(1)
        emit_attn_quarter(0, 0, 0)
        emit_attn_quarter(0, 0, 1)
        emit_k(2)
        emit_v(2)
        emit_attn_quarter(0, 0, 2)
        emit_k(3)
        emit_v(3)
        emit_attn_quarter(0, 0, 3)
        emit_q(1)
        for hp in (1, 2, 3):
            for gp in range(4):
                emit_attn_quarter(hp, 0, gp)
        emit_norm(0)
        emit_attn_quarter(0, 1, 0)
        emit_attn_quarter(0, 1, 1)
        emit_attn_quarter(0, 1, 2)
        emit_out(0)
        emit_attn_quarter(0, 1, 3)
        emit_out(1)
        emit_attn_quarter(1, 1, 0)
        emit_attn_quarter(1, 1, 1)
        emit_attn_quarter(1, 1, 2)
        emit_out(2)
        emit_attn_quarter(1, 1, 3)
        emit_out(3)
        for gp in range(4):
            emit_attn_quarter(2, 1, gp)
        for gp in range(4):
            emit_attn_quarter(3, 1, gp)
        emit_norm(1)
        for m in (4, 5, 6, 7):
            emit_out(m)

    nc.finalize()
    return nc


def _get_program():
    if "nc" not in _CACHE:
        _CACHE["nc"] = _build_program()
    return _CACHE["nc"]


def _host_inputs(x, Wq, bq, Wk, bk, Wv, bv, Wo, bo):
    f32 = np.float32
    bf = ml_dtypes.bfloat16
    f8 = ml_dtypes.float8_e4m3

    def wprep(W, dtype):
        # [EMB_in, EMB_out] -> [P, KT4, EMB_out] (kt-tiled transpose)
        wT = np.asarray(W, f32).T.reshape(KT4, P, EMB)
        return np.ascontiguousarray(wT.transpose(1, 0, 2)).astype(dtype)

    wq2 = wprep(Wq, f8)
    wk2 = wprep(Wk, f8)
    wv2 = wprep(Wv, bf)
    wo2 = wprep(Wo, bf)
    bq2 = np.ascontiguousarray(np.asarray(bq, f32).reshape(KT4, P).T)
    bk2 = np.ascontiguousarray(np.asarray(bk, f32).reshape(KT4, P).T)
    bvr = np.ascontiguousarray(np.tile(np.asarray(bv, f32), (P, 1)))
    bor = np.ascontiguousarray(np.tile(np.asarray(bo, f32), (P, 1)))
    sel = np.zeros((P, KT4, P), f32)
    for et in range(KT4):
        for m in range(P):
            sel[et * 2 + m // D, et, m] = 1.0

    shared = dict(wq=wq2, wk=wk2, wv=wv2, wo=wo2, bq2=bq2, bk2=bk2,
                  bvr=bvr, bor=bor, sel=sel.astype(bf))
    x = np.asarray(x, f32)
    in_maps = []
    for c in range(NCORES):
        b, hf = c // 2, c % 2
        xb = x[b]
        # queries first; key order is irrelevant as long as K and V agree
        xr = np.concatenate(
            [xb[hf * HALF:(hf + 1) * HALF], xb[(1 - hf) * HALF:(2 - hf) * HALF]], 0
        )
        xTf = np.ascontiguousarray(xr.T).reshape(KT4, P, N).transpose(1, 0, 2)
        xTc = np.ascontiguousarray(xTf)
        in_maps.append(dict(shared, xT=xTc.astype(bf), x8=xTc.astype(f8)))
    return in_maps


def kernel(x, Wq, bq, Wk, bk, Wv, bv, Wo, bo, _trace=False, _trace_cores=None):
    from concourse.bass_utils import run_bass_kernel_spmd

    nc = _get_program()
    in_maps = _host_inputs(x, Wq, bq, Wk, bk, Wv, bv, Wo, bo)
    res = run_bass_kernel_spmd(
        nc, in_maps, list(range(NCORES)), trace=_trace,
        trace_cores=_trace_cores,
    )
    out = np.empty((B, N, EMB), np.float32)
    for c in range(NCORES):
        b, hf = c // 2, c % 2
        out[b, hf * HALF:(hf + 1) * HALF] = np.asarray(
            res.results[c]["y"], np.float32
        )
    if _trace:
        _CACHE["last_results"] = res
    return out
